# revision 35
# baseline (speedup 1.0000x reference)
"""Additive-attention score kernel (GNN message passing) for TRN2, 8 NeuronCores.

Reference math (per batch b):
    q[i] = i_em[b,i,:] @ a_w[:D]
    k[j] = i_em[b,j,:] @ a_w[D:]
    s[i,j] = q[i] + k[j] + a_b
    alphas = softmax_j(leaky_relu(s, 0.01))        -> [B, N, N, 1]

Sharding: data-parallel over batch B=16 across 8 cores (2 batches/core).

Per-core pipeline, balanced across all four compute engines so each sits just
under the ~31 us DMA roofline (10.5 MB of HBM traffic per core):
  - GpSimd: q/k elementwise products, k row partition_broadcast, and
    normalize_recip for a subset of row-tiles.
  - VectorE: q/k row reductions, k-row psum->sbuf copies, bf16
    leaky-relu (add / mul / max) for most row-tiles, normalize for the rest.
  - ScalarE: Exp(t - 8) with fused per-row accumulation for every tile
    (softmax is shift-invariant), plus Lrelu (with the q bias folded in) for
    ACT_LRELU tiles per batch; ops grouped by function to avoid
    activation-table thrash.
  - PE: eight tiny [128,1] -> [1,128] transposes per batch for the k row.
Scores for the VectorE path are bf16 (|y| <= ~7, so absolute error ~0.02
worst case -> well inside the 2e-2 gate); the ScalarE path stays fp32.
"""

import numpy as np

import concourse.bacc as bacc
import concourse.bass as bass
import concourse.tile as tile
from concourse import mybir
from concourse.bass_utils import run_bass_kernel_spmd
from concourse.masks import make_identity

P = 128
B, N, D = 16, 1024, 256
NCORES = 8
BPC = B // NCORES          # batches per core
NT = N // P                # row tiles per batch
NEG_SLOPE = 0.01
EXP_SHIFT = -8.0

ACT_LRELU = 2              # row-tiles per batch with lrelu on ScalarE
DVE_NORM = 6               # row-tiles per batch normalized on VectorE (rest GpSimd)
Q_MULT_POOL = False        # q products on GpSimd instead of VectorE
ACT_CHAIN = True           # force ScalarE stream order (groups table loads)
BIG_BUFS = 4               # depth for the per-tile score/exp/out pools

F32 = mybir.dt.float32
BF16 = mybir.dt.bfloat16
ts = bass.ts


def _body(nc, tc, i_em, a_w, a_b, out):
    with (
        tc.tile_pool(name="consts", bufs=1) as consts,
        tc.tile_pool(name="xin", bufs=2) as xin,
        tc.tile_pool(name="kprep", bufs=2) as kprep,
        tc.tile_pool(name="scratch", bufs=4) as scratch_p,
        tc.tile_pool(name="qcol", bufs=16) as qcol_p,
        tc.tile_pool(name="ytile", bufs=BIG_BUFS) as yp,
        tc.tile_pool(name="ttile", bufs=BIG_BUFS) as tp,
        tc.tile_pool(name="etile", bufs=BIG_BUFS) as ep,
        tc.tile_pool(name="otile", bufs=BIG_BUFS) as op_,
        tc.tile_pool(name="small", bufs=12) as small,
        tc.tile_pool(name="psum_t", bufs=2, space="PSUM") as psum_t,
    ):
        identity = consts.tile([P, P], F32)
        make_identity(nc, identity)

        aw_ap = a_w[:]
        aw_b = consts.tile([P, 2 * D], F32)
        nc.sync.dma_start(
            out=aw_b,
            in_=bass.AP(tensor=aw_ap.tensor, offset=aw_ap.offset, ap=[[0, P], [1, 2 * D]]),
        )
        ab_ap = a_b[:]
        b_col = consts.tile([P, 1], F32)
        nc.sync.dma_start(
            out=b_col,
            in_=bass.AP(tensor=ab_ap.tensor, offset=ab_ap.offset, ap=[[0, P], [1, 1]]),
        )
        shift_col = consts.tile([P, 1], F32)
        nc.gpsimd.memset(shift_col, EXP_SHIFT)

        act_chain = []

        # issue every input DMA up front; 2 halves per batch so q/k work can
        # start after the first half lands
        x_alls = []
        for b in range(BPC):
            x_all = xin.tile([P, NT, D], F32)
            x_src = i_em[b].rearrange("(t p) d -> p t d", p=P)
            nc.sync.dma_start(out=x_all[:, 0 : NT // 2, :], in_=x_src[:, 0 : NT // 2, :])
            nc.sync.dma_start(out=x_all[:, NT // 2 :, :], in_=x_src[:, NT // 2 :, :])
            x_alls.append(x_all)

        for b in range(BPC):
            x_all = x_alls[b]

            # ---- q columns (DVE mult) and k columns (GpSimd mult), one fused
            # reduce; k tile immediately biased, transposed, broadcast ----
            k_bcast = kprep.tile([P, N], BF16, tag="kb")
            q_cols = []
            for t in range(NT):
                scr = scratch_p.tile([P, 2, D], F32, tag="s")
                qmul_eng = nc.gpsimd if Q_MULT_POOL else nc.vector
                qmul_eng.tensor_mul(scr[:, 0, :], x_all[:, t, :], aw_b[:, 0:D])
                nc.gpsimd.tensor_mul(scr[:, 1, :], x_all[:, t, :], aw_b[:, D : 2 * D])
                qk_t = qcol_p.tile([P, 2], F32)
                nc.vector.reduce_sum(qk_t, scr, axis=mybir.AxisListType.X)
                q_cols.append(qk_t)
                nc.gpsimd.tensor_scalar_add(
                    qk_t[:, 1:2], qk_t[:, 1:2], b_col[:, 0:1]
                )
                kT_ps = psum_t.tile([1, P], F32)
                nc.tensor.transpose(kT_ps, qk_t[:, 1:2], identity)
                kT_sb = kprep.tile([1, P], BF16, tag="kT")
                nc.vector.tensor_copy(kT_sb, kT_ps)
                nc.gpsimd.partition_broadcast(k_bcast[:, ts(t, P)], kT_sb)

            # ---- per-tile: lrelu'd scores -> exp(+sums) -> normalize -> DMA.
            # The ScalarE stream is explicitly chained (order-only deps) as
            # [Lrelu x ACT_LRELU, Exp x 8] per batch so Bacc inserts exactly
            # one activation-table load per group instead of thrashing.
            def exp_norm_store(t, t_t):
                e_t = ep.tile([P, N], F32)
                sums = small.tile([P, 1], F32)
                act = nc.scalar.activation(
                    out=e_t,
                    in_=t_t,
                    func=mybir.ActivationFunctionType.Exp,
                    bias=shift_col[:, 0:1],
                    scale=1.0,
                    accum_out=sums,
                )
                act_chain.append(act)
                o_t = op_.tile([P, N], F32)
                if t >= NT - DVE_NORM:
                    recip = small.tile([P, 1], F32, tag="recip")
                    nc.vector.reciprocal(recip, sums)
                    nc.vector.tensor_scalar_mul(o_t, e_t, recip)
                else:
                    nc.gpsimd.normalize_recip(o_t, e_t, sums)
                nc.sync.dma_start(out=out[b][ts(t, P), :], in_=o_t)

            # ScalarE stream per batch: [Lrelu x A, Exp x 8] -> 2 table loads
            n_dve = NT - ACT_LRELU
            act_tiles = []
            for t in range(n_dve, NT):
                t_t = tp.tile([P, N], F32, tag="tact")
                act = nc.scalar.activation(
                    out=t_t,
                    in_=k_bcast,
                    func=mybir.ActivationFunctionType.Lrelu,
                    bias=q_cols[t][:, 0:1],
                    scale=1.0,
                    alpha=NEG_SLOPE,
                )
                act_chain.append(act)
                act_tiles.append(t_t)
            for t in range(n_dve):
                y_t = yp.tile([P, N], BF16, tag="y")
                nc.vector.tensor_scalar_add(y_t, k_bcast, q_cols[t][:, 0:1])
                m_t = scratch_p.tile([P, N], BF16, tag="m")
                nc.vector.tensor_scalar_mul(m_t, y_t, NEG_SLOPE)
                t_t = tp.tile([P, N], BF16, tag="tdve")
                nc.vector.tensor_max(t_t, y_t, m_t)
                exp_norm_store(t, t_t)
            for i, t in enumerate(range(n_dve, NT)):
                exp_norm_store(t, act_tiles[i])

        if ACT_CHAIN:
            from concourse.tile_rust import add_dep_helper

            for a, b_ in zip(act_chain, act_chain[1:]):
                add_dep_helper(
                    getattr(b_, "ins", b_),
                    getattr(a, "ins", a),
                    sync=False,
                    reason="scalar-engine stream order (table-load grouping)",
                )


def _build_nc():
    nc = bacc.Bacc()
    i_em = nc.declare_dram_parameter("i_em", [BPC, N, D], F32, isOutput=False)
    a_w = nc.declare_dram_parameter("a_w", [2 * D], F32, isOutput=False)
    a_b = nc.declare_dram_parameter("a_b", [1], F32, isOutput=False)
    out = nc.declare_dram_parameter("out", [BPC, N, N], F32, isOutput=True)
    with tile.TileContext(nc) as tc:
        _body(nc, tc, i_em, a_w, a_b, out)
    nc.compile()
    return nc


_NC = None


def _get_nc():
    global _NC
    if _NC is None:
        _NC = _build_nc()
    return _NC


def _in_maps(i_em, a_w, a_b):
    i_em = np.ascontiguousarray(np.asarray(i_em, dtype=np.float32))
    a_w = np.ascontiguousarray(np.asarray(a_w, dtype=np.float32))
    a_b = np.ascontiguousarray(np.asarray(a_b, dtype=np.float32))
    return [
        {"i_em": i_em[c * BPC : (c + 1) * BPC], "a_w": a_w, "a_b": a_b}
        for c in range(NCORES)
    ]


def run_spmd(i_em, a_w, a_b, **kwargs):
    """Run on 8 cores; returns (full_output, BassKernelResults)."""
    nc = _get_nc()
    r = run_bass_kernel_spmd(nc, _in_maps(i_em, a_w, a_b), list(range(NCORES)), **kwargs)
    out = np.concatenate([m["out"] for m in r.results], axis=0)
    return out.reshape(B, N, N, 1), r


def kernel(i_em, a_w, a_b):
    out, _ = run_spmd(i_em, a_w, a_b)
    return out


# revision 37
# speedup vs baseline: 1.0363x; 1.0363x over previous
"""Additive-attention score kernel (GNN message passing) for TRN2, 8 NeuronCores.

Reference math (per batch b):
    q[i] = i_em[b,i,:] @ a_w[:D]
    k[j] = i_em[b,j,:] @ a_w[D:]
    s[i,j] = q[i] + k[j] + a_b
    alphas = softmax_j(leaky_relu(s, 0.01))        -> [B, N, N, 1]

Sharding: data-parallel over batch B=16 across 8 cores (2 batches/core).

Per-core pipeline, balanced across all four compute engines so each sits just
under the ~31 us DMA roofline (10.5 MB of HBM traffic per core):
  - GpSimd: q/k elementwise products, k row partition_broadcast, and
    normalize_recip for a subset of row-tiles.
  - VectorE: q/k row reductions, k-row psum->sbuf copies, bf16
    leaky-relu (add / mul / max) for most row-tiles, normalize for the rest.
  - ScalarE: Exp(t - 8) with fused per-row accumulation for every tile
    (softmax is shift-invariant), plus Lrelu (with the q bias folded in) for
    ACT_LRELU tiles per batch; ops grouped by function to avoid
    activation-table thrash.
  - PE: eight tiny [128,1] -> [1,128] transposes per batch for the k row.
Scores for the VectorE path are bf16 (|y| <= ~7, so absolute error ~0.02
worst case -> well inside the 2e-2 gate); the ScalarE path stays fp32.
"""

import numpy as np

import concourse.bacc as bacc
import concourse.bass as bass
import concourse.tile as tile
from concourse import mybir
from concourse.bass_utils import run_bass_kernel_spmd
from concourse.masks import make_identity

P = 128
B, N, D = 16, 1024, 256
NCORES = 8
BPC = B // NCORES          # batches per core
NT = N // P                # row tiles per batch
NEG_SLOPE = 0.01
EXP_SHIFT = -8.0

ACT_LRELU = (4, 1)         # row-tiles per batch with lrelu on ScalarE (batch 0
                           # heavier: frees VectorE for batch 1's q/k prologue
                           # during batch 0's score window)
DVE_NORM = 7               # row-tiles per batch normalized on VectorE (rest GpSimd)
Q_MULT_POOL = False        # q products on GpSimd instead of VectorE
ACT_CHAIN = True           # force ScalarE stream order (groups table loads)
BIG_BUFS = 4               # depth for the per-tile score/exp/out pools

F32 = mybir.dt.float32
BF16 = mybir.dt.bfloat16
ts = bass.ts


def _body(nc, tc, i_em, a_w, a_b, out):
    with (
        tc.tile_pool(name="consts", bufs=1) as consts,
        tc.tile_pool(name="xin", bufs=2) as xin,
        tc.tile_pool(name="kprep", bufs=2) as kprep,
        tc.tile_pool(name="scratch", bufs=4) as scratch_p,
        tc.tile_pool(name="qcol", bufs=16) as qcol_p,
        tc.tile_pool(name="ytile", bufs=BIG_BUFS) as yp,
        tc.tile_pool(name="ttile", bufs=BIG_BUFS) as tp,
        tc.tile_pool(name="etile", bufs=BIG_BUFS) as ep,
        tc.tile_pool(name="otile", bufs=BIG_BUFS) as op_,
        tc.tile_pool(name="small", bufs=12) as small,
        tc.tile_pool(name="psum_t", bufs=2, space="PSUM") as psum_t,
    ):
        identity = consts.tile([P, P], F32)
        make_identity(nc, identity)

        aw_ap = a_w[:]
        aw_b = consts.tile([P, 2 * D], F32)
        nc.sync.dma_start(
            out=aw_b,
            in_=bass.AP(tensor=aw_ap.tensor, offset=aw_ap.offset, ap=[[0, P], [1, 2 * D]]),
        )
        ab_ap = a_b[:]
        b_col = consts.tile([P, 1], F32)
        nc.sync.dma_start(
            out=b_col,
            in_=bass.AP(tensor=ab_ap.tensor, offset=ab_ap.offset, ap=[[0, P], [1, 1]]),
        )
        shift_col = consts.tile([P, 1], F32)
        nc.gpsimd.memset(shift_col, EXP_SHIFT)

        act_chain = []

        # issue every input DMA up front; 2 halves per batch so q/k work can
        # start after the first half lands
        x_alls = []
        for b in range(BPC):
            x_all = xin.tile([P, NT, D], F32)
            x_src = i_em[b].rearrange("(t p) d -> p t d", p=P)
            nc.sync.dma_start(out=x_all[:, 0 : NT // 2, :], in_=x_src[:, 0 : NT // 2, :])
            nc.sync.dma_start(out=x_all[:, NT // 2 :, :], in_=x_src[:, NT // 2 :, :])
            x_alls.append(x_all)

        for b in range(BPC):
            x_all = x_alls[b]

            # ---- q columns (DVE mult) and k columns (GpSimd mult), one fused
            # reduce; k tile immediately biased, transposed, broadcast ----
            k_bcast = kprep.tile([P, N], BF16, tag="kb")
            q_cols = []
            for t in range(NT):
                scr = scratch_p.tile([P, 2, D], F32, tag="s")
                qmul_eng = nc.gpsimd if Q_MULT_POOL else nc.vector
                qmul_eng.tensor_mul(scr[:, 0, :], x_all[:, t, :], aw_b[:, 0:D])
                nc.gpsimd.tensor_mul(scr[:, 1, :], x_all[:, t, :], aw_b[:, D : 2 * D])
                qk_t = qcol_p.tile([P, 2], F32)
                nc.vector.reduce_sum(qk_t, scr, axis=mybir.AxisListType.X)
                q_cols.append(qk_t)
                nc.gpsimd.tensor_scalar_add(
                    qk_t[:, 1:2], qk_t[:, 1:2], b_col[:, 0:1]
                )
                kT_ps = psum_t.tile([1, P], F32)
                nc.tensor.transpose(kT_ps, qk_t[:, 1:2], identity)
                kT_sb = kprep.tile([1, P], BF16, tag="kT")
                nc.vector.tensor_copy(kT_sb, kT_ps)
                nc.gpsimd.partition_broadcast(k_bcast[:, ts(t, P)], kT_sb)

            # ---- per-tile: lrelu'd scores -> exp(+sums) -> normalize -> DMA.
            # The ScalarE stream is explicitly chained (order-only deps) as
            # [Lrelu x ACT_LRELU, Exp x 8] per batch so Bacc inserts exactly
            # one activation-table load per group instead of thrashing.
            def exp_norm_store(t, t_t):
                e_t = ep.tile([P, N], F32)
                sums = small.tile([P, 1], F32)
                act = nc.scalar.activation(
                    out=e_t,
                    in_=t_t,
                    func=mybir.ActivationFunctionType.Exp,
                    bias=shift_col[:, 0:1],
                    scale=1.0,
                    accum_out=sums,
                )
                act_chain.append(act)
                o_t = op_.tile([P, N], F32)
                if t >= NT - DVE_NORM:
                    recip = small.tile([P, 1], F32, tag="recip")
                    nc.vector.reciprocal(recip, sums)
                    nc.vector.tensor_scalar_mul(o_t, e_t, recip)
                else:
                    nc.gpsimd.normalize_recip(o_t, e_t, sums)
                nc.sync.dma_start(out=out[b][ts(t, P), :], in_=o_t)

            # ScalarE stream per batch: [Lrelu x A, Exp x 8] -> 2 table loads
            a_lrelu = ACT_LRELU[b] if isinstance(ACT_LRELU, (list, tuple)) else ACT_LRELU
            n_dve = NT - a_lrelu
            act_tiles = []
            for t in range(n_dve, NT):
                t_t = tp.tile([P, N], F32, tag="tact")
                act = nc.scalar.activation(
                    out=t_t,
                    in_=k_bcast,
                    func=mybir.ActivationFunctionType.Lrelu,
                    bias=q_cols[t][:, 0:1],
                    scale=1.0,
                    alpha=NEG_SLOPE,
                )
                act_chain.append(act)
                act_tiles.append(t_t)
            for t in range(n_dve):
                y_t = yp.tile([P, N], BF16, tag="y")
                nc.vector.tensor_scalar_add(y_t, k_bcast, q_cols[t][:, 0:1])
                m_t = scratch_p.tile([P, N], BF16, tag="m")
                nc.vector.tensor_scalar_mul(m_t, y_t, NEG_SLOPE)
                t_t = tp.tile([P, N], BF16, tag="tdve")
                nc.vector.tensor_max(t_t, y_t, m_t)
                exp_norm_store(t, t_t)
            for i, t in enumerate(range(n_dve, NT)):
                exp_norm_store(t, act_tiles[i])

        if ACT_CHAIN:
            from concourse.tile_rust import add_dep_helper

            for a, b_ in zip(act_chain, act_chain[1:]):
                add_dep_helper(
                    getattr(b_, "ins", b_),
                    getattr(a, "ins", a),
                    sync=False,
                    reason="scalar-engine stream order (table-load grouping)",
                )


def _build_nc():
    nc = bacc.Bacc()
    i_em = nc.declare_dram_parameter("i_em", [BPC, N, D], F32, isOutput=False)
    a_w = nc.declare_dram_parameter("a_w", [2 * D], F32, isOutput=False)
    a_b = nc.declare_dram_parameter("a_b", [1], F32, isOutput=False)
    out = nc.declare_dram_parameter("out", [BPC, N, N], F32, isOutput=True)
    with tile.TileContext(nc) as tc:
        _body(nc, tc, i_em, a_w, a_b, out)
    nc.compile()
    return nc


_NC = None


def _get_nc():
    global _NC
    if _NC is None:
        _NC = _build_nc()
    return _NC


def _in_maps(i_em, a_w, a_b):
    i_em = np.ascontiguousarray(np.asarray(i_em, dtype=np.float32))
    a_w = np.ascontiguousarray(np.asarray(a_w, dtype=np.float32))
    a_b = np.ascontiguousarray(np.asarray(a_b, dtype=np.float32))
    return [
        {"i_em": i_em[c * BPC : (c + 1) * BPC], "a_w": a_w, "a_b": a_b}
        for c in range(NCORES)
    ]


def run_spmd(i_em, a_w, a_b, **kwargs):
    """Run on 8 cores; returns (full_output, BassKernelResults)."""
    nc = _get_nc()
    r = run_bass_kernel_spmd(nc, _in_maps(i_em, a_w, a_b), list(range(NCORES)), **kwargs)
    out = np.concatenate([m["out"] for m in r.results], axis=0)
    return out.reshape(B, N, N, 1), r


def kernel(i_em, a_w, a_b):
    out, _ = run_spmd(i_em, a_w, a_b)
    return out


# revision 45
# speedup vs baseline: 1.1140x; 1.0750x over previous
"""Additive-attention score kernel (GNN message passing) for TRN2, 8 NeuronCores.

Reference math (per batch b):
    q[i] = i_em[b,i,:] @ a_w[:D]
    k[j] = i_em[b,j,:] @ a_w[D:]
    s[i,j] = q[i] + k[j] + a_b
    alphas = softmax_j(leaky_relu(s, 0.01))        -> [B, N, N, 1]

Sharding: data-parallel over batch B=16 across 8 cores (2 batches/core).

Per-core pipeline, balanced across all four compute engines so each sits just
under the ~31 us DMA roofline (10.5 MB of HBM traffic per core):
  - GpSimd: q/k elementwise products, k row partition_broadcast, and
    normalize_recip for a subset of row-tiles.
  - VectorE: q/k row reductions, k-row psum->sbuf copies, bf16
    leaky-relu (add / mul / max) for most row-tiles, normalize for the rest.
  - ScalarE: Exp(t - 8) with fused per-row accumulation for every tile
    (softmax is shift-invariant), plus Lrelu (with the q bias folded in) for
    ACT_LRELU tiles per batch; ops grouped by function to avoid
    activation-table thrash.
  - PE: eight tiny [128,1] -> [1,128] transposes per batch for the k row.
Scores for the VectorE path are bf16 (|y| <= ~7, so absolute error ~0.02
worst case -> well inside the 2e-2 gate); the ScalarE path stays fp32.
"""

import numpy as np

import concourse.bacc as bacc
import concourse.bass as bass
import concourse.tile as tile
from concourse import mybir
from concourse.bass_utils import run_bass_kernel_spmd
from concourse.masks import make_identity

P = 128
B, N, D = 16, 1024, 256
NCORES = 8
BPC = B // NCORES          # batches per core
NT = N // P                # row tiles per batch
NEG_SLOPE = 0.01
EXP_SHIFT = -8.0

ACT_LRELU = (4, 1)
                           # heavier: frees VectorE for batch 1's q/k prologue
                           # during batch 0's score window)
DVE_NORM = 4
Q_MULT_POOL = False        # q products on GpSimd instead of VectorE
EXPS_FIRST = False         # emit trio-tile exp chains before the Lrelu block
K_MULT_POOL_B1 = False     # batch-1 k products on GpSimd (frees VectorE mid-window)
ACT_CHAIN = True           # force ScalarE stream order (groups table loads)
BIG_BUFS = 4               # depth for the per-tile score/exp/out pools

F32 = mybir.dt.float32
BF16 = mybir.dt.bfloat16
ts = bass.ts


def _body(nc, tc, i_em, a_w, a_b, out):
    with (
        tc.tile_pool(name="consts", bufs=1) as consts,
        tc.tile_pool(name="xin", bufs=2) as xin,
        tc.tile_pool(name="kprep", bufs=2) as kprep,
        tc.tile_pool(name="scratch", bufs=4) as scratch_p,
        tc.tile_pool(name="qcol", bufs=16) as qcol_p,
        tc.tile_pool(name="ytile", bufs=BIG_BUFS) as yp,
        tc.tile_pool(name="ttile", bufs=BIG_BUFS) as tp,
        tc.tile_pool(name="etile", bufs=BIG_BUFS) as ep,
        tc.tile_pool(name="otile", bufs=BIG_BUFS) as op_,
        tc.tile_pool(name="small", bufs=12) as small,
        tc.tile_pool(name="psum_t", bufs=2, space="PSUM") as psum_t,
    ):
        identity = consts.tile([P, P], F32)
        make_identity(nc, identity)

        aw_ap = a_w[:]
        aw_b = consts.tile([P, 2 * D], F32)
        nc.sync.dma_start(
            out=aw_b,
            in_=bass.AP(tensor=aw_ap.tensor, offset=aw_ap.offset, ap=[[0, P], [1, 2 * D]]),
        )
        ab_ap = a_b[:]
        b_col = consts.tile([P, 1], F32)
        nc.sync.dma_start(
            out=b_col,
            in_=bass.AP(tensor=ab_ap.tensor, offset=ab_ap.offset, ap=[[0, P], [1, 1]]),
        )
        shift_col = consts.tile([P, 1], F32)
        nc.gpsimd.memset(shift_col, EXP_SHIFT)

        act_chain = []

        # issue every input DMA up front; 4 chunks per batch so q/k work can
        # start as soon as the first quarter lands
        x_alls = []
        for b in range(BPC):
            x_all = xin.tile([P, NT, D], F32)
            x_src = i_em[b].rearrange("(t p) d -> p t d", p=P)
            for c in range(0, NT, 2):
                nc.sync.dma_start(
                    out=x_all[:, c : c + 2, :], in_=x_src[:, c : c + 2, :]
                )
            x_alls.append(x_all)

        for b in range(BPC):
            x_all = x_alls[b]

            # ---- k path first (latency-critical: everything waits on the
            # full k_bcast). All on VectorE for the tightest per-tile cadence;
            # ScalarE (idle now) moves each k row off PSUM via Copy, which is
            # in every activation table so it never forces a table reload. ----
            k_bcast = kprep.tile([P, N], BF16, tag="kb")
            for t in range(NT):
                scr_k = scratch_p.tile([P, D], F32, tag="sk")
                keng = nc.gpsimd if (K_MULT_POOL_B1 and b == 1) else nc.vector
                keng.tensor_mul(scr_k, x_all[:, t, :], aw_b[:, D : 2 * D])
                k_t = qcol_p.tile([P, 1], F32, tag="kt")
                nc.vector.reduce_sum(k_t, scr_k, axis=mybir.AxisListType.X)
                nc.vector.tensor_scalar_add(k_t, k_t, b_col[:, 0:1])
                kT_ps = psum_t.tile([1, P], F32)
                nc.tensor.transpose(kT_ps, k_t, identity)
                kT_sb = kprep.tile([1, P], BF16, tag="kT")
                nc.scalar.activation(
                    out=kT_sb,
                    in_=kT_ps,
                    func=mybir.ActivationFunctionType.Copy,
                    bias=0.0,
                    scale=1.0,
                )
                nc.gpsimd.partition_broadcast(k_bcast[:, ts(t, P)], kT_sb)

            # ---- q columns (slack: tile t's q is only needed by its own
            # score tile). Products on GpSimd, reductions on VectorE. ----
            q_cols = []
            for t in range(NT):
                scr_q = scratch_p.tile([P, D], F32, tag="sq")
                nc.gpsimd.tensor_mul(scr_q, x_all[:, t, :], aw_b[:, 0:D])
                q_t = qcol_p.tile([P, 1], F32, tag="qt")
                nc.vector.reduce_sum(q_t, scr_q, axis=mybir.AxisListType.X)
                q_cols.append(q_t)

            # ---- per-tile: lrelu'd scores -> exp(+sums) -> normalize -> DMA.
            # The ScalarE stream is explicitly chained (order-only deps) as
            # [Lrelu x ACT_LRELU, Exp x 8] per batch so Bacc inserts exactly
            # one activation-table load per group instead of thrashing.
            def exp_norm_store(t, t_t):
                e_t = ep.tile([P, N], F32)
                sums = small.tile([P, 1], F32)
                act = nc.scalar.activation(
                    out=e_t,
                    in_=t_t,
                    func=mybir.ActivationFunctionType.Exp,
                    bias=shift_col[:, 0:1],
                    scale=1.0,
                    accum_out=sums,
                )
                act_chain.append(act)
                o_t = op_.tile([P, N], F32)
                if t >= NT - DVE_NORM:
                    recip = small.tile([P, 1], F32, tag="recip")
                    nc.vector.reciprocal(recip, sums)
                    nc.vector.tensor_scalar_mul(o_t, e_t, recip)
                else:
                    nc.gpsimd.normalize_recip(o_t, e_t, sums)
                nc.sync.dma_start(out=out[b][ts(t, P), :], in_=o_t)

            # ScalarE stream per batch: [Lrelu x A, Exp x 8] -> 2 table loads
            a_lrelu = ACT_LRELU[b] if isinstance(ACT_LRELU, (list, tuple)) else ACT_LRELU
            n_dve = NT - a_lrelu
            def emit_lrelus():
                act_tiles = []
                for t in range(n_dve, NT):
                    t_t = tp.tile([P, N], F32, tag="tact")
                    act = nc.scalar.activation(
                        out=t_t,
                        in_=k_bcast,
                        func=mybir.ActivationFunctionType.Lrelu,
                        bias=q_cols[t][:, 0:1],
                        scale=1.0,
                        alpha=NEG_SLOPE,
                    )
                    act_chain.append(act)
                    act_tiles.append(t_t)
                return act_tiles

            act_tiles = None if EXPS_FIRST else emit_lrelus()
            for t in range(n_dve):
                y_t = yp.tile([P, N], BF16, tag="y")
                nc.vector.tensor_scalar_add(y_t, k_bcast, q_cols[t][:, 0:1])
                m_t = scratch_p.tile([P, N], BF16, tag="m")
                nc.vector.tensor_scalar_mul(m_t, y_t, NEG_SLOPE)
                t_t = tp.tile([P, N], BF16, tag="tdve")
                nc.vector.tensor_max(t_t, y_t, m_t)
                exp_norm_store(t, t_t)
            if act_tiles is None:
                act_tiles = emit_lrelus()
            for i, t in enumerate(range(n_dve, NT)):
                exp_norm_store(t, act_tiles[i])

        if ACT_CHAIN:
            from concourse.tile_rust import add_dep_helper

            for a, b_ in zip(act_chain, act_chain[1:]):
                add_dep_helper(
                    getattr(b_, "ins", b_),
                    getattr(a, "ins", a),
                    sync=False,
                    reason="scalar-engine stream order (table-load grouping)",
                )


def _build_nc():
    nc = bacc.Bacc()
    i_em = nc.declare_dram_parameter("i_em", [BPC, N, D], F32, isOutput=False)
    a_w = nc.declare_dram_parameter("a_w", [2 * D], F32, isOutput=False)
    a_b = nc.declare_dram_parameter("a_b", [1], F32, isOutput=False)
    out = nc.declare_dram_parameter("out", [BPC, N, N], F32, isOutput=True)
    with tile.TileContext(nc) as tc:
        _body(nc, tc, i_em, a_w, a_b, out)
    nc.compile()
    return nc


_NC = None


def _get_nc():
    global _NC
    if _NC is None:
        _NC = _build_nc()
    return _NC


def _in_maps(i_em, a_w, a_b):
    i_em = np.ascontiguousarray(np.asarray(i_em, dtype=np.float32))
    a_w = np.ascontiguousarray(np.asarray(a_w, dtype=np.float32))
    a_b = np.ascontiguousarray(np.asarray(a_b, dtype=np.float32))
    return [
        {"i_em": i_em[c * BPC : (c + 1) * BPC], "a_w": a_w, "a_b": a_b}
        for c in range(NCORES)
    ]


def run_spmd(i_em, a_w, a_b, **kwargs):
    """Run on 8 cores; returns (full_output, BassKernelResults)."""
    nc = _get_nc()
    r = run_bass_kernel_spmd(nc, _in_maps(i_em, a_w, a_b), list(range(NCORES)), **kwargs)
    out = np.concatenate([m["out"] for m in r.results], axis=0)
    return out.reshape(B, N, N, 1), r


def kernel(i_em, a_w, a_b):
    out, _ = run_spmd(i_em, a_w, a_b)
    return out
